# revision 43
# baseline (speedup 1.0000x reference)
"""Distributed Trainium2 (8 NeuronCores) kernel for nn_AdaptiveAttention.

Reference computation (b=2, n=2048, d=1024, 16 heads x 64):
    qkv = x @ W_qkv; q,k,v = split(qkv)
    attn = softmax(mask(q k^T / sqrt(dh)))
    out  = (attn @ v) @ W_out + b_out

Sharding: core c in [0,8) handles batch b = c//4 and head group g = c%4
(heads 4g..4g+3).  Data parallel over b, tensor parallel over heads.

Schedule (per core): a short pre-round projects q/k for the first head
pair, then paired attention rounds cover the (pair, i-block) combos in
order (p0,i0) (p1,i0) (p0,i1) (p1,i1a) (p1,i1b); in every round the two
concurrent heads' score matmuls land on complementary PE row-tiles
(0,0)/(64,0) since head_dim=64.  Remaining projections (v just-in-time,
later q/k groups) are woven into rounds 0-2 using the shared score-PSUM
pool.  The last round is split into two 512-i half-rounds so the
second-to-last output chunk's exchange overlaps the final half-round.

Back end (no cross-core attention exchange): each head's attention is
normalized LOCALLY (softmax sums ride as a ones-column in v_aug;
reciprocal rows are DMA-packed and broadcast across partitions with a
tiny K=2 selector matmul).  The output projection is ROW-PARALLEL: each
core contracts its 256 head-dims against its W_out row-slice over ALL
2048 i rows, producing a bf16 partial [2048, 1024] that a 4-rank
ReduceScatter(add) per 512-i chunk reduces straight into output shards.
All normalize+project work is woven into later rounds; the kernel tail
is only the last half-round's window plus the final chunk's exchange.

The mask ships as fp8 (exact for 0/1) and is cast to bf16 by the gpsimd
software-DGE on the way into SBUF, halving its HBM traffic.

Numerics: all matmuls bf16 operands with fp32 PSUM accumulation.
Softmax runs without max-subtraction (scores are O(1) by construction)
as exp(s) * mask.
"""

import numpy as np
import ml_dtypes

import concourse.bass as bass
import concourse.tile as tile
from concourse import bacc, mybir
from concourse import bass_utils

BF16 = ml_dtypes.bfloat16

B = 2
N = 2048
D = 1024
HEADS = 16
HD = 64  # head dim
SCALE = HD ** -0.5
N_CORES = 8
HPC = 4  # heads per core
IB = 1024  # full i-block size (one i-half)
NJ = N // 128  # 16 j-chunks

_cached_nc = None
_last_in_maps = None
_last_res = None


def _build():
    nc = bacc.Bacc("TRN2", target_bir_lowering=False, debug=False,
                   num_devices=N_CORES)

    f32 = mybir.dt.float32
    bf = mybir.dt.bfloat16
    fp8 = mybir.dt.float8e4

    xt = nc.dram_tensor("xt", [D, N], bf, kind="ExternalInput")
    wqkv = nc.dram_tensor("wqkv", [D, 768], bf, kind="ExternalInput")
    # mask is exactly 0/1 so it ships as fp8 (half the HBM traffic) and
    # the gpsimd software-DGE casts it to bf16 on the way into SBUF
    maskt = nc.dram_tensor("maskt", [N, N], fp8, kind="ExternalInput")
    # W_out rows for this core's 4 heads, packed [128, 2*1024]:
    # col 1024*p + c  <->  W_out[256*g + 128*p + partition, c]
    wout = nc.dram_tensor("wout", [128, 2 * D], bf, kind="ExternalInput")
    # partition-broadcast selector: e2[s, r] = 1 iff s == r // 64
    e2 = nc.dram_tensor("e2", [2, 128], bf, kind="ExternalInput")
    out = nc.dram_tensor("out", [N // 4, D], bf, kind="ExternalOutput")

    with tile.TileContext(nc) as tc:
        with (
            tc.tile_pool(name="res", bufs=1) as res,
            tc.tile_pool(name="dram", bufs=1, space="DRAM") as dram,
            tc.tile_pool(name="pe", bufs=6) as p_e,
            tc.tile_pool(name="pp", bufs=6) as p_p,
            tc.tile_pool(name="pao", bufs=3) as p_ao,
            tc.tile_pool(name="ptb", bufs=3) as p_tb,
            tc.tile_pool(name="prec", bufs=4) as p_rec,
            tc.tile_pool(name="prc2", bufs=3) as p_rc2,
            tc.tile_pool(name="pan", bufs=3) as p_an,
            tc.tile_pool(name="ost", bufs=3) as po,
            tc.tile_pool(name="prs", bufs=2) as p_rs,
            tc.tile_pool(name="pps", bufs=2, space="PSUM") as pp_s,
            tc.tile_pool(name="ppa1", bufs=1, space="PSUM") as pp_a1,
            tc.tile_pool(name="ppa2", bufs=1, space="PSUM") as pp_a2,
        ):
            # resident tensors
            # qkt: [qT01 | qT23 | kT01 | kT23], each [128, 2048] bf16
            qkt = res.tile([128, 4 * N], bf)
            # v_aug: per j-chunk jc block of 260 cols: 4x(64 v cols + ones)
            v_aug = res.tile([128, NJ * 260], bf)
            # mask, one tile per j-chunk for fine-grained load deps
            mts = [res.tile([128, N], bf, name=f"mt{jc}") for jc in range(NJ)]
            wout_sb = res.tile([128, 2 * D], bf)
            e2_sb = res.tile([2, 128], bf)
            ones2 = res.tile([2, IB], bf)

            # row-parallel out-proj partials, one dram chunk per 512 i
            # rows; ReduceScatter(add) within each batch's 4-core group
            # reduces chunk g and scatters 128-row shards
            part = [dram.tile([512, D], bf, name=f"part{g}")
                    for g in range(2)]
            rso = [dram.tile([128, D], bf, name=f"rso{g}") for g in range(2)]
            # the whole second i-half exchanges as ONE collective: the
            # per-collective fixed cost and inter-collective gap on the
            # single CC stream outweigh finer-grained overlap at the tail
            part23 = dram.tile([2 * 512, D], bf, name="part23")
            rso23 = dram.tile([256, D], bf, name="rso23")

            nc.vector.memset(v_aug[:], 1.0)
            nc.vector.memset(ones2[:], 1.0)

            # tiny warm-up ReduceScatter: absorbs the first-collective
            # channel setup cost during the load/projection phase
            cwu_in = dram.tile([8, 16], bf, name="cwu_in")
            cwu_out = dram.tile([2, 16], bf, name="cwu_out")
            wu_sb = res.tile([8, 16], bf)
            nc.vector.memset(wu_sb[:], 1.0)
            nc.sync.dma_start(cwu_in[:, :], wu_sb[:])
            nc.gpsimd.collective_compute(
                "ReduceScatter", mybir.AluOpType.add,
                replica_groups=[[0, 1, 2, 3], [4, 5, 6, 7]],
                ins=[cwu_in[:].opt()], outs=[cwu_out[:].opt()],
            )

            # ---- round bodies -------------------------------------
            # hl = head-local index (0..3) = 2*pair + hh
            # i_off/i_w: absolute i column offset and width of the round
            def sc_iter(hl, i_off, i_w, jc):
                pair, hh = hl // 2, hl % 2
                q_off = N * pair
                k_off = N * (2 + pair)
                s_ps = pp_s.tile([128, i_w], f32, name="s_ps", tag="mm")
                for ih in range(i_w // 512):
                    nc.tensor.matmul(
                        s_ps[:, 512 * ih:512 * ih + 512],
                        qkt[64 * hh:64 * hh + 64,
                            k_off + 128 * jc:k_off + 128 * jc + 128],
                        qkt[64 * hh:64 * hh + 64,
                            q_off + i_off + 512 * ih:
                            q_off + i_off + 512 * ih + 512],
                        start=True, stop=True,
                    )
                return s_ps

            def ep_iter(i_off, i_w, jc, s_ps):
                e_t = p_e.tile([128, IB], bf, name="e_t", tag="e_t")
                nc.scalar.activation(
                    e_t[:, 0:i_w], s_ps[:],
                    mybir.ActivationFunctionType.Exp)
                p_t = p_p.tile([128, IB], bf, name="p_t", tag="p_t")
                nc.vector.tensor_mul(
                    p_t[:, 0:i_w], e_t[:, 0:i_w],
                    mts[jc][:, i_off:i_off + i_w])
                return p_t

            def av_iter(hl, i_w, jc, acc, p_t):
                for ih in range(i_w // 512):
                    nc.tensor.matmul(
                        acc[:, 512 * ih:512 * ih + 512],
                        v_aug[:, 260 * jc + 65 * hl:
                              260 * jc + 65 * hl + 65],
                        p_t[:, 512 * ih:512 * ih + 512],
                        start=(jc == 0), stop=(jc == NJ - 1),
                    )

            def round_tail(i_w, accA, accB):
                """Evacuate both heads' raw attention into a packed
                [128, i_w] tile (head hh on partitions 64hh..) and the
                softmax-sum reciprocals into a packed [2, i_w] tile."""
                # evacuate the two softmax-sum rows (bf16), pack them
                # onto partitions 0-1, and take the reciprocal on the
                # otherwise-idle Pool engine (DVE's Newton reciprocal on
                # a 1-partition row costs ~6.5us of pacer-engine time)
                rA = p_rec.tile([65, IB], bf, name="rA", tag="rec")
                rB = p_rec.tile([65, IB], bf, name="rB", tag="rec")
                nc.vector.tensor_copy(rA[64:65, 0:i_w], accA[64:65, :])
                nc.vector.tensor_copy(rB[64:65, 0:i_w], accB[64:65, :])
                at2 = p_ao.tile([128, IB], bf, name="at2", tag="at2")
                nc.vector.tensor_copy(at2[0:64, 0:i_w], accA[0:64, :])
                tmpB = p_tb.tile([64, IB], bf, name="tmpB", tag="tb")
                nc.vector.tensor_copy(tmpB[:, 0:i_w], accB[0:64, :])
                # partition move: SBUF->SBUF DMA into upper half
                nc.sync.dma_start(at2[64:128, 0:i_w], tmpB[:, 0:i_w])
                sm2 = p_rc2.tile([2, IB], bf, name="sm2", tag="sm2")
                nc.sync.dma_start(sm2[0:1, 0:i_w], rA[64:65, 0:i_w])
                nc.sync.dma_start(sm2[1:2, 0:i_w], rB[64:65, 0:i_w])
                return at2, sm2

            def tail_recip_start(sm2, i_w):
                # deferred off the round boundary and chunked into 256-
                # wide pieces: a monolithic Newton reciprocal would hold
                # the in-order DVE queue for ~7us and starve the next
                # round's mask multiplies
                rc2 = p_rc2.tile([2, IB], bf, name="rc2", tag="rc2")
                return rc2

            def tail_recip_piece(sm2, rc2, q):
                lo = 256 * q
                with nc.allow_low_precision(reason="softmax recip bf16"):
                    nc.vector.reciprocal(rc2[:, lo:lo + 256],
                                         sm2[:, lo:lo + 256])

            def run_round(pair, i_off, i_w, weave):
                hlA, hlB = 2 * pair, 2 * pair + 1
                accA = pp_a1.tile([65, i_w], f32, name="acc", tag="acc")
                accB = pp_a2.tile([65, i_w], f32, name="acc", tag="acc")
                pA = ep_iter(i_off, i_w, 0, sc_iter(hlA, i_off, i_w, 0))
                pB = ep_iter(i_off, i_w, 0, sc_iter(hlB, i_off, i_w, 0))
                for jc in range(NJ):
                    # interleave each head's next-chunk score/exp with the
                    # other head's current av so PSUM slots free earlier
                    # and cross-engine handoffs overlap
                    if jc + 1 < NJ:
                        pA_n = ep_iter(i_off, i_w, jc + 1,
                                       sc_iter(hlA, i_off, i_w, jc + 1))
                    av_iter(hlA, i_w, jc, accA, pA)
                    if jc + 1 < NJ:
                        pB_n = ep_iter(i_off, i_w, jc + 1,
                                       sc_iter(hlB, i_off, i_w, jc + 1))
                    av_iter(hlB, i_w, jc, accB, pB)
                    for job in weave.get(jc, ()):
                        job()
                    pA, pB = pA_n, pB_n
                return round_tail(i_w, accA, accB)

            # at_n entries: (tile, base) -- absolute i maps to tile col
            # (i - base)
            def win_norm(p, at2, rc2, i_w, at_n, base):
                """Broadcast the pair's softmax reciprocals across the
                128 head-dim partitions (K=2 selector matmul) and
                normalize its raw attention."""
                bc2 = pp_s.tile([128, i_w], f32, name="bc2", tag="mm")
                for ih in range(i_w // 512):
                    nc.tensor.matmul(bc2[:, 512 * ih:512 * ih + 512],
                                     e2_sb[:],
                                     rc2[:, 512 * ih:512 * ih + 512],
                                     start=True, stop=True)
                an = p_an.tile([128, IB], bf, name="an", tag="an")
                nc.vector.tensor_mul(an[:, 0:i_w], at2[:, 0:i_w], bc2[:])
                at_n[p] = (an, base)

            def win_piece(g, it, at_n):
                """One 128-i tile of the row-parallel out-projection;
                absolute i rows [512g + 128it, +128)."""
                i_abs = 512 * g + 128 * it
                for nh in range(2):
                    ps = pp_s.tile([128, 512], f32, name="ps_o", tag="mm")
                    for p in range(2):
                        an, base = at_n[p]
                        i0 = i_abs - base
                        nc.tensor.matmul(
                            ps[:],
                            an[:, i0:i0 + 128],
                            wout_sb[:, D * p + 512 * nh:
                                    D * p + 512 * nh + 512],
                            start=(p == 0), stop=(p == 1),
                        )
                    ot = po.tile([128, 512], bf, name="ot", tag="ot")
                    # Act is the exp-paced bottleneck engine; keep all
                    # PSUM evacuations on the half-idle DVE
                    nc.vector.tensor_copy(ot[:], ps[:])
                    if g < 2:
                        dst = part[g][128 * it:128 * it + 128,
                                      512 * nh:512 * nh + 512]
                    else:
                        row = 512 * (g - 2) + 128 * it
                        dst = part23[row:row + 128,
                                     512 * nh:512 * nh + 512]
                    nc.sync.dma_start(dst, ot[:])

            def win_fire(g):
                nc.gpsimd.collective_compute(
                    "ReduceScatter", mybir.AluOpType.add,
                    replica_groups=[[0, 1, 2, 3], [4, 5, 6, 7]],
                    ins=[part[g][:].opt()], outs=[rso[g][:].opt()],
                )

            def win_fire23():
                nc.gpsimd.collective_compute(
                    "ReduceScatter", mybir.AluOpType.add,
                    replica_groups=[[0, 1, 2, 3], [4, 5, 6, 7]],
                    ins=[part23[:].opt()], outs=[rso23[:].opt()],
                )

            def drain_chunk(g):
                # read the reduced shard back (sync-queue DMAs have
                # reliably enforced collective-completion waits) and
                # store it to the output
                rsb = p_rs.tile([128, D], bf, name="rsb", tag="rs")
                nc.sync.dma_start(rsb[:], rso[g][:, :])
                nc.sync.dma_start(out[128 * g:128 * g + 128, :], rsb[:])

            def drain_chunk23():
                for q in range(2):
                    rsb = p_rs.tile([128, D], bf, name="rsb", tag="rs")
                    nc.sync.dma_start(rsb[:], rso23[128 * q:128 * q + 128, :])
                    nc.sync.dma_start(out[256 + 128 * q:384 + 128 * q, :],
                                      rsb[:])

            with (
                tc.tile_pool(name="ph0", bufs=1) as p0,
            ):
                xtr = [p0.tile([128, N], bf, name=f"xtr{k}")
                       for k in range(8)]
                wr = [p0.tile([128, 768], bf, name=f"wr{k}")
                      for k in range(8)]
                # x and qkv weights round-robin over ALL THREE DMA
                # rings (sync + scalar HWDGE, gpsimd SWDGE) -- the load
                # phase is ring-bandwidth-bound, and the SWDGE ring is
                # otherwise idle until the fp8 masks queue behind
                qs = [nc.sync, nc.scalar, nc.gpsimd]
                for k in range(8):
                    qs[(2 * k) % 3].dma_start(
                        xtr[k][:], xt[128 * k:128 * (k + 1), :])
                    qs[(2 * k + 1) % 3].dma_start(
                        wr[k][:], wqkv[128 * k:128 * (k + 1), :])
                for jc in range(NJ):
                    nc.gpsimd.dma_start(
                        mts[jc][:], maskt[128 * jc:128 * (jc + 1), :])
                nc.scalar.dma_start(wout_sb[:], wout[:, :])
                nc.scalar.dma_start(e2_sb[:], e2[:, :])

                def proj_qk_group(t_i, nb):
                    wcol = 128 * t_i
                    ps = pp_s.tile([128, 512], f32, name="ps_qk", tag="mm")
                    for k in range(8):
                        nc.tensor.matmul(
                            ps[:],
                            wr[k][:, wcol:wcol + 128],
                            xtr[k][:, 512 * nb:512 * nb + 512],
                            start=(k == 0), stop=(k == 7),
                        )
                    nc.vector.tensor_copy(
                        qkt[:, N * t_i + 512 * nb:N * t_i + 512 * nb + 512],
                        ps[:])

                def proj_v_group(jc):
                    ps = pp_s.tile([128, 256], f32, name="ps_v", tag="mm")
                    for k in range(8):
                        nc.tensor.matmul(
                            ps[:],
                            xtr[k][:, 128 * jc:128 * jc + 128],
                            wr[k][:, 512:768],
                            start=(k == 0), stop=(k == 7),
                        )
                    for h in range(4):
                        nc.vector.tensor_copy(
                            v_aug[:, 260 * jc + 65 * h:260 * jc + 65 * h + 64],
                            ps[:, 64 * h:64 * h + 64])

                # pre-round: only what round 0 jc0 needs (q01 i-half 0,
                # first k chunk, first v chunk); the rest weaves into
                # rounds 0-2 ahead of first use
                proj_qk_group(0, 0)
                proj_qk_group(0, 1)
                proj_qk_group(2, 0)
                proj_v_group(0)

                # R0: pair 0, i-half 0; weave v just-in-time, the rest
                # of kT01 (nb_k first read at jc 4k), and pair 1's
                # round-1 start (q23 i-half0 + first k23 chunk)
                w0 = {jc: [lambda jc=jc: proj_v_group(jc + 1)]
                      for jc in range(NJ - 1)}
                w0[2] = w0[2] + [lambda: proj_qk_group(2, 1)]
                w0[4] = w0[4] + [lambda: proj_qk_group(1, 0)]
                w0[6] = w0[6] + [lambda: proj_qk_group(2, 2)]
                w0[8] = w0[8] + [lambda: proj_qk_group(1, 1)]
                w0[10] = w0[10] + [lambda: proj_qk_group(2, 3)]
                w0[12] = w0[12] + [lambda: proj_qk_group(3, 0)]
                at2_00, sm2_00 = run_round(0, 0, IB, w0)

                # R1: pair 1, i-half 0; weave the remaining k23 chunks
                # and q01's i-half 1
                rcs = {}
                w1 = {2 * i + 1: [lambda t=t, nb=nb: proj_qk_group(t, nb)]
                      for i, (t, nb) in enumerate(
                          [(3, 1), (0, 2), (3, 2), (0, 3), (3, 3)])}
                w1[2] = [lambda: rcs.__setitem__(
                    0, tail_recip_start(sm2_00, IB))] + []
                for q in range(4):
                    w1[4 + 2 * q] = w1.get(4 + 2 * q, []) + [
                        lambda q=q: tail_recip_piece(sm2_00, rcs[0], q)]
                at2_10, sm2_10 = run_round(1, 0, IB, w1)

                # W1 (normalize + project + exchange i-half 0) weaves
                # into round 2 alongside pair 1's i-half-1 q projections
                # (the last xtr/wr consumers); RS chunks 0-1 overlap
                # rounds 2-3
                at_n0 = [None, None]
                w2 = {
                    0: [lambda: rcs.__setitem__(
                            1, tail_recip_start(sm2_10, IB)),
                        lambda: tail_recip_piece(sm2_10, rcs[1], 0),
                        lambda: win_norm(0, at2_00, rcs[0], IB, at_n0, 0)],
                    1: [lambda: tail_recip_piece(sm2_10, rcs[1], 1),
                        lambda: tail_recip_piece(sm2_10, rcs[1], 2)],
                    2: [lambda: tail_recip_piece(sm2_10, rcs[1], 3),
                        lambda: win_norm(1, at2_10, rcs[1], IB, at_n0, 0)],
                    11: [lambda: proj_qk_group(1, 2)],
                    13: [lambda: proj_qk_group(1, 3)],
                }
                for ic in range(2):
                    for it in range(4):
                        jobs = [lambda ic=ic, it=it:
                                win_piece(ic, it, at_n0)]
                        if it == 3:
                            jobs.append(lambda ic=ic: win_fire(ic))
                        w2[3 + 4 * ic + it] = jobs

                # R2: pair 0, i-half 1
                at2_01, sm2_01 = run_round(0, 1 * IB, IB, w2)

            # projections done: xtr/wr freed
            # R3a: pair 1, i [1024:1536); weave pair 0's i-half-1
            # normalize so only pair 1's remains after each half-round
            at_n1 = [None, None]
            w3a = {
                0: [lambda: rcs.__setitem__(
                    2, tail_recip_start(sm2_01, IB))],
                1: [lambda: tail_recip_piece(sm2_01, rcs[2], 0),
                    lambda: tail_recip_piece(sm2_01, rcs[2], 1)],
                2: [lambda: tail_recip_piece(sm2_01, rcs[2], 2),
                    lambda: tail_recip_piece(sm2_01, rcs[2], 3)],
                3: [lambda: win_norm(0, at2_01, rcs[2], IB, at_n1, IB)],
            }
            at2_1a, sm2_1a = run_round(1, 1 * IB, 512, w3a)

            # W2a: normalize pair1[1024:1536), project + fire chunk 2;
            # weaves into R3b so RS chunk 2 overlaps the last half-round
            w3b = {
                0: [lambda: rcs.__setitem__(
                    3, tail_recip_start(sm2_1a, 512))],
                1: [lambda: tail_recip_piece(sm2_1a, rcs[3], 0),
                    lambda: tail_recip_piece(sm2_1a, rcs[3], 1)],
                2: [lambda: win_norm(1, at2_1a, rcs[3], 512, at_n1, IB)],
                8: [lambda: drain_chunk(0)],
                10: [lambda: drain_chunk(1)],
            }
            for it in range(4):
                w3b[3 + it] = [lambda it=it: win_piece(2, it, at_n1)]

            # R3b: pair 1, i [1536:2048)
            at2_1b, sm2_1b = run_round(1, 1 * IB + 512, 512, w3b)

            # tail: last chunk's window + exchange + drains
            rc2_1b = tail_recip_start(sm2_1b, 512)
            tail_recip_piece(sm2_1b, rc2_1b, 0)
            tail_recip_piece(sm2_1b, rc2_1b, 1)
            win_norm(1, at2_1b, rc2_1b, 512, at_n1, IB + 512)
            for it in range(4):
                win_piece(3, it, at_n1)
            win_fire23()
            drain_chunk23()

    nc.compile()
    return nc


def _get_nc():
    global _cached_nc
    if _cached_nc is None:
        _cached_nc = _build()
    return _cached_nc


def kernel(x, mask, W_qkv, W_out, b_out):
    x = np.asarray(x, dtype=np.float32)
    mask = np.asarray(mask)
    W_qkv = np.asarray(W_qkv, dtype=np.float32)
    W_out = np.asarray(W_out, dtype=np.float32)
    b_out = np.asarray(b_out, dtype=np.float32)

    nc = _get_nc()

    FP8 = ml_dtypes.float8_e4m3
    maskt_fp8 = np.ascontiguousarray(mask.reshape(N, N).T).astype(FP8)
    # partition-broadcast selector for the softmax reciprocals
    e2 = np.zeros((2, 128), dtype=np.float32)
    e2[0, 0:64] = 1.0
    e2[1, 64:128] = 1.0
    e2 = np.ascontiguousarray(e2).astype(BF16)

    in_maps = []
    for c in range(N_CORES):
        b = c // 4
        g = c % 4
        hs = slice(g * HPC * HD, (g + 1) * HPC * HD)  # 256 cols of this core
        wq = W_qkv[:, 0 * D:1 * D][:, hs] * np.float32(SCALE)
        wk = W_qkv[:, 1 * D:2 * D][:, hs]
        wv = W_qkv[:, 2 * D:3 * D][:, hs]
        wqkv_c = np.ascontiguousarray(
            np.concatenate([wq, wk, wv], axis=1)).astype(BF16)
        xt_c = np.ascontiguousarray(x[b].T).astype(BF16)
        # W_out rows for this core's heads, packed [128, 2048]
        wrows = W_out[256 * g:256 * (g + 1), :]
        wout_c = np.ascontiguousarray(
            np.concatenate([wrows[0:128, :], wrows[128:256, :]],
                           axis=1)).astype(BF16)
        in_maps.append({
            "xt": xt_c,
            "wqkv": wqkv_c,
            "maskt": maskt_fp8,
            "wout": wout_c,
            "e2": e2,
        })

    global _last_in_maps, _last_res
    _last_in_maps = in_maps

    res = bass_utils.run_bass_kernel_spmd(
        nc, in_maps, core_ids=list(range(N_CORES)))
    _last_res = res

    # core r = 4b + rr holds, for chunk g, reduced output rows
    # [512g + 128rr, 512g + 128rr + 128) of batch b at out[128g:...]
    out_full = np.empty((B, N, D), dtype=np.float32)
    for c in range(N_CORES):
        b = c // 4
        rr = c % 4
        core_out = res.results[c]["out"].astype(np.float32)
        for g in range(2):
            out_full[b, 512 * g + 128 * rr:512 * g + 128 * rr + 128, :] = \
                core_out[128 * g:128 * g + 128]
        out_full[b, 1024 + 256 * rr:1024 + 256 * rr + 256, :] = \
            core_out[256:512]
    out_full += b_out
    return out_full


# revision 44
# speedup vs baseline: 1.1234x; 1.1234x over previous
"""Distributed Trainium2 (8 NeuronCores) kernel for nn_AdaptiveAttention.

Reference computation (b=2, n=2048, d=1024, 16 heads x 64):
    qkv = x @ W_qkv; q,k,v = split(qkv)
    attn = softmax(mask(q k^T / sqrt(dh)))
    out  = (attn @ v) @ W_out + b_out

Sharding: core c in [0,8) handles batch b = c//4 and head group g = c%4
(heads 4g..4g+3).  Data parallel over b, tensor parallel over heads.

Schedule (per core): a short pre-round projects q/k for the first head
pair, then paired attention rounds cover the (pair, i-block) combos in
order (p0,i0) (p1,i0) (p0,i1) (p1,i1a) (p1,i1b); in every round the two
concurrent heads' score matmuls land on complementary PE row-tiles
(0,0)/(64,0) since head_dim=64.  Remaining projections (v just-in-time,
later q/k groups) are woven into rounds 0-2 using the shared score-PSUM
pool.  The last round is split into two 512-i half-rounds so the
second-to-last output chunk's exchange overlaps the final half-round.

Back end (no cross-core attention exchange): each head's attention is
normalized LOCALLY (softmax sums ride as a ones-column in v_aug;
reciprocal rows are DMA-packed and broadcast across partitions with a
tiny K=2 selector matmul).  The output projection is ROW-PARALLEL: each
core contracts its 256 head-dims against its W_out row-slice over ALL
2048 i rows, producing a bf16 partial [2048, 1024] that a 4-rank
ReduceScatter(add) per 512-i chunk reduces straight into output shards.
All normalize+project work is woven into later rounds; the kernel tail
is only the last half-round's window plus the final chunk's exchange.

The mask ships as fp8 (exact for 0/1) and is cast to bf16 by the gpsimd
software-DGE on the way into SBUF, halving its HBM traffic.

Numerics: all matmuls bf16 operands with fp32 PSUM accumulation.
Softmax runs without max-subtraction (scores are O(1) by construction)
as exp(s) * mask.
"""

import numpy as np
import ml_dtypes

import concourse.bass as bass
import concourse.tile as tile
from concourse import bacc, mybir
from concourse import bass_utils

BF16 = ml_dtypes.bfloat16

B = 2
N = 2048
D = 1024
HEADS = 16
HD = 64  # head dim
SCALE = HD ** -0.5
N_CORES = 8
HPC = 4  # heads per core
IB = 1024  # full i-block size (one i-half)
NJ = N // 128  # 16 j-chunks

_cached_nc = None
_last_in_maps = None
_last_res = None


def _build():
    nc = bacc.Bacc("TRN2", target_bir_lowering=False, debug=False,
                   num_devices=N_CORES)

    f32 = mybir.dt.float32
    bf = mybir.dt.bfloat16
    fp8 = mybir.dt.float8e4

    xt = nc.dram_tensor("xt", [D, N], bf, kind="ExternalInput")
    wqkv = nc.dram_tensor("wqkv", [D, 768], bf, kind="ExternalInput")
    # mask is exactly 0/1 so it ships as fp8 (half the HBM traffic) and
    # the gpsimd software-DGE casts it to bf16 on the way into SBUF
    maskt = nc.dram_tensor("maskt", [N, N], fp8, kind="ExternalInput")
    # W_out rows for this core's 4 heads, packed [128, 2*1024]:
    # col 1024*p + c  <->  W_out[256*g + 128*p + partition, c]
    wout = nc.dram_tensor("wout", [128, 2 * D], bf, kind="ExternalInput")
    # partition-broadcast selector: e2[s, r] = 1 iff s == r // 64
    e2 = nc.dram_tensor("e2", [2, 128], bf, kind="ExternalInput")
    out = nc.dram_tensor("out", [N // 4, D], bf, kind="ExternalOutput")

    with tile.TileContext(nc) as tc:
        with (
            tc.tile_pool(name="res", bufs=1) as res,
            tc.tile_pool(name="dram", bufs=1, space="DRAM") as dram,
            tc.tile_pool(name="pe", bufs=6) as p_e,
            tc.tile_pool(name="pp", bufs=6) as p_p,
            tc.tile_pool(name="pao", bufs=3) as p_ao,
            tc.tile_pool(name="ptb", bufs=3) as p_tb,
            tc.tile_pool(name="prec", bufs=4) as p_rec,
            tc.tile_pool(name="prc2", bufs=3) as p_rc2,
            tc.tile_pool(name="pan", bufs=3) as p_an,
            tc.tile_pool(name="ost", bufs=3) as po,
            tc.tile_pool(name="prs", bufs=2) as p_rs,
            tc.tile_pool(name="pps", bufs=2, space="PSUM") as pp_s,
            tc.tile_pool(name="ppa1", bufs=1, space="PSUM") as pp_a1,
            tc.tile_pool(name="ppa2", bufs=1, space="PSUM") as pp_a2,
        ):
            # resident tensors
            # qkt: [qT01 | qT23 | kT01 | kT23], each [128, 2048] bf16
            qkt = res.tile([128, 4 * N], bf)
            # v_aug: per j-chunk jc block of 260 cols: 4x(64 v cols + ones)
            v_aug = res.tile([128, NJ * 260], bf)
            # mask, one tile per j-chunk for fine-grained load deps
            mts = [res.tile([128, N], bf, name=f"mt{jc}") for jc in range(NJ)]
            wout_sb = res.tile([128, 2 * D], bf)
            e2_sb = res.tile([2, 128], bf)
            ones2 = res.tile([2, IB], bf)

            # row-parallel out-proj partials, one dram chunk per 512 i
            # rows; ReduceScatter(add) within each batch's 4-core group
            # reduces chunk g and scatters 128-row shards
            part = [dram.tile([512, D], bf, name=f"part{g}")
                    for g in range(4)]
            rso = [dram.tile([128, D], bf, name=f"rso{g}") for g in range(4)]

            nc.vector.memset(v_aug[:], 1.0)
            nc.vector.memset(ones2[:], 1.0)

            # tiny warm-up ReduceScatter: absorbs the first-collective
            # channel setup cost during the load/projection phase
            cwu_in = dram.tile([8, 16], bf, name="cwu_in")
            cwu_out = dram.tile([2, 16], bf, name="cwu_out")
            wu_sb = res.tile([8, 16], bf)
            nc.vector.memset(wu_sb[:], 1.0)
            nc.sync.dma_start(cwu_in[:, :], wu_sb[:])
            nc.gpsimd.collective_compute(
                "ReduceScatter", mybir.AluOpType.add,
                replica_groups=[[0, 1, 2, 3], [4, 5, 6, 7]],
                ins=[cwu_in[:].opt()], outs=[cwu_out[:].opt()],
            )

            # ---- round bodies -------------------------------------
            # hl = head-local index (0..3) = 2*pair + hh
            # i_off/i_w: absolute i column offset and width of the round
            def sc_iter(hl, i_off, i_w, jc):
                pair, hh = hl // 2, hl % 2
                q_off = N * pair
                k_off = N * (2 + pair)
                s_ps = pp_s.tile([128, i_w], f32, name="s_ps", tag="mm")
                for ih in range(i_w // 512):
                    nc.tensor.matmul(
                        s_ps[:, 512 * ih:512 * ih + 512],
                        qkt[64 * hh:64 * hh + 64,
                            k_off + 128 * jc:k_off + 128 * jc + 128],
                        qkt[64 * hh:64 * hh + 64,
                            q_off + i_off + 512 * ih:
                            q_off + i_off + 512 * ih + 512],
                        start=True, stop=True,
                    )
                return s_ps

            def ep_iter(i_off, i_w, jc, s_ps):
                e_t = p_e.tile([128, IB], bf, name="e_t", tag="e_t")
                nc.scalar.activation(
                    e_t[:, 0:i_w], s_ps[:],
                    mybir.ActivationFunctionType.Exp)
                p_t = p_p.tile([128, IB], bf, name="p_t", tag="p_t")
                nc.vector.tensor_mul(
                    p_t[:, 0:i_w], e_t[:, 0:i_w],
                    mts[jc][:, i_off:i_off + i_w])
                return p_t

            def av_iter(hl, i_w, jc, acc, p_t):
                for ih in range(i_w // 512):
                    nc.tensor.matmul(
                        acc[:, 512 * ih:512 * ih + 512],
                        v_aug[:, 260 * jc + 65 * hl:
                              260 * jc + 65 * hl + 65],
                        p_t[:, 512 * ih:512 * ih + 512],
                        start=(jc == 0), stop=(jc == NJ - 1),
                    )

            def round_tail(i_w, accA, accB):
                """Evacuate both heads' raw attention into a packed
                [128, i_w] tile (head hh on partitions 64hh..) and the
                softmax-sum reciprocals into a packed [2, i_w] tile."""
                # evacuate the two softmax-sum rows (bf16), pack them
                # onto partitions 0-1, and take the reciprocal on the
                # otherwise-idle Pool engine (DVE's Newton reciprocal on
                # a 1-partition row costs ~6.5us of pacer-engine time)
                rA = p_rec.tile([65, IB], bf, name="rA", tag="rec")
                rB = p_rec.tile([65, IB], bf, name="rB", tag="rec")
                nc.vector.tensor_copy(rA[64:65, 0:i_w], accA[64:65, :])
                nc.vector.tensor_copy(rB[64:65, 0:i_w], accB[64:65, :])
                at2 = p_ao.tile([128, IB], bf, name="at2", tag="at2")
                nc.vector.tensor_copy(at2[0:64, 0:i_w], accA[0:64, :])
                tmpB = p_tb.tile([64, IB], bf, name="tmpB", tag="tb")
                nc.vector.tensor_copy(tmpB[:, 0:i_w], accB[0:64, :])
                # partition move: SBUF->SBUF DMA into upper half
                nc.sync.dma_start(at2[64:128, 0:i_w], tmpB[:, 0:i_w])
                sm2 = p_rc2.tile([2, IB], bf, name="sm2", tag="sm2")
                nc.sync.dma_start(sm2[0:1, 0:i_w], rA[64:65, 0:i_w])
                nc.sync.dma_start(sm2[1:2, 0:i_w], rB[64:65, 0:i_w])
                return at2, sm2

            def tail_recip_start(sm2, i_w):
                # deferred off the round boundary and chunked into 256-
                # wide pieces: a monolithic Newton reciprocal would hold
                # the in-order DVE queue for ~7us and starve the next
                # round's mask multiplies
                rc2 = p_rc2.tile([2, IB], bf, name="rc2", tag="rc2")
                return rc2

            def tail_recip_piece(sm2, rc2, q):
                lo = 256 * q
                with nc.allow_low_precision(reason="softmax recip bf16"):
                    nc.vector.reciprocal(rc2[:, lo:lo + 256],
                                         sm2[:, lo:lo + 256])

            def run_round(pair, i_off, i_w, weave):
                hlA, hlB = 2 * pair, 2 * pair + 1
                accA = pp_a1.tile([65, i_w], f32, name="acc", tag="acc")
                accB = pp_a2.tile([65, i_w], f32, name="acc", tag="acc")
                pA = ep_iter(i_off, i_w, 0, sc_iter(hlA, i_off, i_w, 0))
                pB = ep_iter(i_off, i_w, 0, sc_iter(hlB, i_off, i_w, 0))
                for jc in range(NJ):
                    # interleave each head's next-chunk score/exp with the
                    # other head's current av so PSUM slots free earlier
                    # and cross-engine handoffs overlap
                    if jc + 1 < NJ:
                        pA_n = ep_iter(i_off, i_w, jc + 1,
                                       sc_iter(hlA, i_off, i_w, jc + 1))
                    av_iter(hlA, i_w, jc, accA, pA)
                    if jc + 1 < NJ:
                        pB_n = ep_iter(i_off, i_w, jc + 1,
                                       sc_iter(hlB, i_off, i_w, jc + 1))
                    av_iter(hlB, i_w, jc, accB, pB)
                    for job in weave.get(jc, ()):
                        job()
                    pA, pB = pA_n, pB_n
                return round_tail(i_w, accA, accB)

            # at_n entries: (tile, base) -- absolute i maps to tile col
            # (i - base)
            def win_norm(p, at2, rc2, i_w, at_n, base):
                """Broadcast the pair's softmax reciprocals across the
                128 head-dim partitions (K=2 selector matmul) and
                normalize its raw attention."""
                bc2 = pp_s.tile([128, i_w], f32, name="bc2", tag="mm")
                for ih in range(i_w // 512):
                    nc.tensor.matmul(bc2[:, 512 * ih:512 * ih + 512],
                                     e2_sb[:],
                                     rc2[:, 512 * ih:512 * ih + 512],
                                     start=True, stop=True)
                an = p_an.tile([128, IB], bf, name="an", tag="an")
                nc.vector.tensor_mul(an[:, 0:i_w], at2[:, 0:i_w], bc2[:])
                at_n[p] = (an, base)

            def win_piece(g, it, at_n):
                """One 128-i tile of the row-parallel out-projection;
                absolute i rows [512g + 128it, +128)."""
                i_abs = 512 * g + 128 * it
                for nh in range(2):
                    ps = pp_s.tile([128, 512], f32, name="ps_o", tag="mm")
                    for p in range(2):
                        an, base = at_n[p]
                        i0 = i_abs - base
                        nc.tensor.matmul(
                            ps[:],
                            an[:, i0:i0 + 128],
                            wout_sb[:, D * p + 512 * nh:
                                    D * p + 512 * nh + 512],
                            start=(p == 0), stop=(p == 1),
                        )
                    ot = po.tile([128, 512], bf, name="ot", tag="ot")
                    # Act is the exp-paced bottleneck engine; keep all
                    # PSUM evacuations on the half-idle DVE
                    nc.vector.tensor_copy(ot[:], ps[:])
                    nc.sync.dma_start(
                        part[g][128 * it:128 * it + 128,
                                512 * nh:512 * nh + 512],
                        ot[:])

            def win_fire(g):
                nc.gpsimd.collective_compute(
                    "ReduceScatter", mybir.AluOpType.add,
                    replica_groups=[[0, 1, 2, 3], [4, 5, 6, 7]],
                    ins=[part[g][:].opt()], outs=[rso[g][:].opt()],
                )

            def drain_chunk(g):
                # read the reduced shard back (sync-queue DMAs have
                # reliably enforced collective-completion waits) and
                # store it to the output
                rsb = p_rs.tile([128, D], bf, name="rsb", tag="rs")
                nc.sync.dma_start(rsb[:], rso[g][:, :])
                nc.sync.dma_start(out[128 * g:128 * g + 128, :], rsb[:])

            with (
                tc.tile_pool(name="ph0", bufs=1) as p0,
            ):
                xtr = [p0.tile([128, N], bf, name=f"xtr{k}")
                       for k in range(8)]
                wr = [p0.tile([128, 768], bf, name=f"wr{k}")
                      for k in range(8)]
                # x and qkv weights round-robin over ALL THREE DMA
                # rings (sync + scalar HWDGE, gpsimd SWDGE) -- the load
                # phase is ring-bandwidth-bound, and the SWDGE ring is
                # otherwise idle until the fp8 masks queue behind
                qs = [nc.sync, nc.scalar, nc.gpsimd]
                for k in range(8):
                    qs[(2 * k) % 3].dma_start(
                        xtr[k][:], xt[128 * k:128 * (k + 1), :])
                    qs[(2 * k + 1) % 3].dma_start(
                        wr[k][:], wqkv[128 * k:128 * (k + 1), :])
                for jc in range(NJ):
                    nc.gpsimd.dma_start(
                        mts[jc][:], maskt[128 * jc:128 * (jc + 1), :])
                nc.scalar.dma_start(wout_sb[:], wout[:, :])
                nc.scalar.dma_start(e2_sb[:], e2[:, :])

                def proj_qk_group(t_i, nb):
                    wcol = 128 * t_i
                    ps = pp_s.tile([128, 512], f32, name="ps_qk", tag="mm")
                    for k in range(8):
                        nc.tensor.matmul(
                            ps[:],
                            wr[k][:, wcol:wcol + 128],
                            xtr[k][:, 512 * nb:512 * nb + 512],
                            start=(k == 0), stop=(k == 7),
                        )
                    nc.vector.tensor_copy(
                        qkt[:, N * t_i + 512 * nb:N * t_i + 512 * nb + 512],
                        ps[:])

                def proj_v_group(jc):
                    ps = pp_s.tile([128, 256], f32, name="ps_v", tag="mm")
                    for k in range(8):
                        nc.tensor.matmul(
                            ps[:],
                            xtr[k][:, 128 * jc:128 * jc + 128],
                            wr[k][:, 512:768],
                            start=(k == 0), stop=(k == 7),
                        )
                    for h in range(4):
                        nc.vector.tensor_copy(
                            v_aug[:, 260 * jc + 65 * h:260 * jc + 65 * h + 64],
                            ps[:, 64 * h:64 * h + 64])

                # pre-round: only what round 0 jc0 needs (q01 i-half 0,
                # first k chunk, first v chunk); the rest weaves into
                # rounds 0-2 ahead of first use
                proj_qk_group(0, 0)
                proj_qk_group(0, 1)
                proj_qk_group(2, 0)
                proj_v_group(0)

                # R0: pair 0, i-half 0; weave v just-in-time, the rest
                # of kT01 (nb_k first read at jc 4k), and pair 1's
                # round-1 start (q23 i-half0 + first k23 chunk)
                w0 = {jc: [lambda jc=jc: proj_v_group(jc + 1)]
                      for jc in range(NJ - 1)}
                w0[2] = w0[2] + [lambda: proj_qk_group(2, 1)]
                w0[4] = w0[4] + [lambda: proj_qk_group(1, 0)]
                w0[6] = w0[6] + [lambda: proj_qk_group(2, 2)]
                w0[8] = w0[8] + [lambda: proj_qk_group(1, 1)]
                w0[10] = w0[10] + [lambda: proj_qk_group(2, 3)]
                w0[12] = w0[12] + [lambda: proj_qk_group(3, 0)]
                at2_00, sm2_00 = run_round(0, 0, IB, w0)

                # R1: pair 1, i-half 0; weave the remaining k23 chunks
                # and q01's i-half 1
                rcs = {}
                w1 = {2 * i + 1: [lambda t=t, nb=nb: proj_qk_group(t, nb)]
                      for i, (t, nb) in enumerate(
                          [(3, 1), (0, 2), (3, 2), (0, 3), (3, 3)])}
                w1[2] = [lambda: rcs.__setitem__(
                    0, tail_recip_start(sm2_00, IB))] + []
                for q in range(4):
                    w1[4 + 2 * q] = w1.get(4 + 2 * q, []) + [
                        lambda q=q: tail_recip_piece(sm2_00, rcs[0], q)]
                at2_10, sm2_10 = run_round(1, 0, IB, w1)

                # W1 (normalize + project + exchange i-half 0) weaves
                # into round 2 alongside pair 1's i-half-1 q projections
                # (the last xtr/wr consumers); RS chunks 0-1 overlap
                # rounds 2-3
                at_n0 = [None, None]
                w2 = {
                    0: [lambda: rcs.__setitem__(
                            1, tail_recip_start(sm2_10, IB)),
                        lambda: tail_recip_piece(sm2_10, rcs[1], 0),
                        lambda: win_norm(0, at2_00, rcs[0], IB, at_n0, 0)],
                    1: [lambda: tail_recip_piece(sm2_10, rcs[1], 1),
                        lambda: tail_recip_piece(sm2_10, rcs[1], 2)],
                    2: [lambda: tail_recip_piece(sm2_10, rcs[1], 3),
                        lambda: win_norm(1, at2_10, rcs[1], IB, at_n0, 0)],
                    11: [lambda: proj_qk_group(1, 2)],
                    13: [lambda: proj_qk_group(1, 3)],
                }
                for ic in range(2):
                    for it in range(4):
                        jobs = [lambda ic=ic, it=it:
                                win_piece(ic, it, at_n0)]
                        if it == 3:
                            jobs.append(lambda ic=ic: win_fire(ic))
                        w2[3 + 4 * ic + it] = jobs

                # R2: pair 0, i-half 1
                at2_01, sm2_01 = run_round(0, 1 * IB, IB, w2)

            # projections done: xtr/wr freed
            # R3a: pair 1, i [1024:1536); weave pair 0's i-half-1
            # normalize so only pair 1's remains after each half-round
            at_n1 = [None, None]
            w3a = {
                0: [lambda: rcs.__setitem__(
                    2, tail_recip_start(sm2_01, IB))],
                1: [lambda: tail_recip_piece(sm2_01, rcs[2], 0),
                    lambda: tail_recip_piece(sm2_01, rcs[2], 1)],
                2: [lambda: tail_recip_piece(sm2_01, rcs[2], 2),
                    lambda: tail_recip_piece(sm2_01, rcs[2], 3)],
                3: [lambda: win_norm(0, at2_01, rcs[2], IB, at_n1, IB)],
            }
            at2_1a, sm2_1a = run_round(1, 1 * IB, 512, w3a)

            # W2a: normalize pair1[1024:1536), project + fire chunk 2;
            # weaves into R3b so RS chunk 2 overlaps the last half-round
            w3b = {
                0: [lambda: rcs.__setitem__(
                    3, tail_recip_start(sm2_1a, 512))],
                1: [lambda: tail_recip_piece(sm2_1a, rcs[3], 0),
                    lambda: tail_recip_piece(sm2_1a, rcs[3], 1)],
                2: [lambda: win_norm(1, at2_1a, rcs[3], 512, at_n1, IB)],
                8: [lambda: drain_chunk(0)],
                10: [lambda: drain_chunk(1)],
            }
            for it in range(4):
                jobs = [lambda it=it: win_piece(2, it, at_n1)]
                if it == 3:
                    jobs.append(lambda: win_fire(2))
                w3b[3 + it] = jobs

            # R3b: pair 1, i [1536:2048)
            at2_1b, sm2_1b = run_round(1, 1 * IB + 512, 512, w3b)

            # tail: last chunk's window + exchange + drains
            rc2_1b = tail_recip_start(sm2_1b, 512)
            tail_recip_piece(sm2_1b, rc2_1b, 0)
            tail_recip_piece(sm2_1b, rc2_1b, 1)
            win_norm(1, at2_1b, rc2_1b, 512, at_n1, IB + 512)
            for it in range(4):
                win_piece(3, it, at_n1)
            win_fire(3)
            drain_chunk(2)
            drain_chunk(3)

    nc.compile()
    return nc


def _get_nc():
    global _cached_nc
    if _cached_nc is None:
        _cached_nc = _build()
    return _cached_nc


def kernel(x, mask, W_qkv, W_out, b_out):
    x = np.asarray(x, dtype=np.float32)
    mask = np.asarray(mask)
    W_qkv = np.asarray(W_qkv, dtype=np.float32)
    W_out = np.asarray(W_out, dtype=np.float32)
    b_out = np.asarray(b_out, dtype=np.float32)

    nc = _get_nc()

    FP8 = ml_dtypes.float8_e4m3
    maskt_fp8 = np.ascontiguousarray(mask.reshape(N, N).T).astype(FP8)
    # partition-broadcast selector for the softmax reciprocals
    e2 = np.zeros((2, 128), dtype=np.float32)
    e2[0, 0:64] = 1.0
    e2[1, 64:128] = 1.0
    e2 = np.ascontiguousarray(e2).astype(BF16)

    in_maps = []
    for c in range(N_CORES):
        b = c // 4
        g = c % 4
        hs = slice(g * HPC * HD, (g + 1) * HPC * HD)  # 256 cols of this core
        wq = W_qkv[:, 0 * D:1 * D][:, hs] * np.float32(SCALE)
        wk = W_qkv[:, 1 * D:2 * D][:, hs]
        wv = W_qkv[:, 2 * D:3 * D][:, hs]
        wqkv_c = np.ascontiguousarray(
            np.concatenate([wq, wk, wv], axis=1)).astype(BF16)
        xt_c = np.ascontiguousarray(x[b].T).astype(BF16)
        # W_out rows for this core's heads, packed [128, 2048]
        wrows = W_out[256 * g:256 * (g + 1), :]
        wout_c = np.ascontiguousarray(
            np.concatenate([wrows[0:128, :], wrows[128:256, :]],
                           axis=1)).astype(BF16)
        in_maps.append({
            "xt": xt_c,
            "wqkv": wqkv_c,
            "maskt": maskt_fp8,
            "wout": wout_c,
            "e2": e2,
        })

    global _last_in_maps, _last_res
    _last_in_maps = in_maps

    res = bass_utils.run_bass_kernel_spmd(
        nc, in_maps, core_ids=list(range(N_CORES)))
    _last_res = res

    # core r = 4b + rr holds, for chunk g, reduced output rows
    # [512g + 128rr, 512g + 128rr + 128) of batch b at out[128g:...]
    out_full = np.empty((B, N, D), dtype=np.float32)
    for c in range(N_CORES):
        b = c // 4
        rr = c % 4
        core_out = res.results[c]["out"].astype(np.float32)
        for g in range(4):
            out_full[b, 512 * g + 128 * rr:512 * g + 128 * rr + 128, :] = \
                core_out[128 * g:128 * g + 128]
    out_full += b_out
    return out_full
